# revision 24
# baseline (speedup 1.0000x reference)
"""Multi-head attention (B=4, S=2048, D=1024, H=16) on 8 Trainium2 cores.

Sharding: data-parallel over the 4 batches x tensor-parallel over 2 groups
of 8 heads. Core c handles batch c//2, head group c%2. Each core computes
its group's slice of the out-projection; the host sums the two partial
outputs per batch.

All matmul operands are bf16 (fp32 PSUM accumulation); rel-err budget is
2e-2 so bf16 rounding (~0.4%) is fine and it halves both PE streaming and
DMA cost vs float32r.

Device-side layout (per core):
  qhT/khT [128, 4, S] bf16 : projections transposed (head-pair dim on
                     partitions: head 2j at partitions 0-63, head 2j+1 at
                     64-127; sequence on free dim).
  scores  : per head pair, key tiles on partitions, 256-query chunks; the
            two heads of a pair issue as back-to-back matmuls on row groups
            (0,0)/(64,0) so they run concurrently in the PE array.
  softmax : exp on ScalarE straight out of PSUM in [128, 3, 2, 256] groups
            (1536 elem/instr); denominators from a ones column appended to
            V during the attn@V accumulation.
  outT [8, 8, 128, 256] f32 : transposed partial out-projection, summed on
            host.
"""
import sys

for _p in ("/opt/trn_rl_repo", "/root/.axon_site/_ro/trn_rl_repo"):
    if _p not in sys.path:
        sys.path.append(_p)

import numpy as np
from ml_dtypes import bfloat16

import concourse.bass as bass
import concourse.tile as tile
from concourse import bacc, mybir
from concourse.bass_utils import run_bass_kernel_spmd

N_CORES = 8
B, S, DIM, H, DK = 4, 2048, 1024, 16, 64
JG = DIM // 2          # head-group width (8 heads x 64)
HPG = 8                # heads per group
BF16 = mybir.dt.bfloat16
F32 = mybir.dt.float32

N_KC = DIM // 128      # contraction chunks for projections
N_JT = JG // 128       # 128-row tiles of the group width
N_SJT = S // 128       # key tiles
QC = 256               # queries per attention chunk
N_QC = S // QC         # attention chunks
SJ_GRP = 2             # key tiles per score/exp group (2 PSUM banks)


def build_program(phases="ABC", unpack_probe=False):
    nc = bacc.Bacc("TRN2", target_bir_lowering=False, debug=False,
                   num_devices=N_CORES)
    xqT = nc.dram_tensor("xqT", [2, N_KC, 128, 1024], BF16,
                         kind="ExternalInput").ap()
    xkT = nc.dram_tensor("xkT", [2, N_KC, 128, 1024], BF16,
                         kind="ExternalInput").ap()
    xvT = nc.dram_tensor("xvT", [2, N_KC, 128, 1024], BF16,
                         kind="ExternalInput").ap()
    wqT = nc.dram_tensor("wqT", [128, N_KC, JG], BF16,
                         kind="ExternalInput").ap()
    wkT = nc.dram_tensor("wkT", [128, N_KC, JG], BF16,
                         kind="ExternalInput").ap()
    wvT = nc.dram_tensor("wvT", [128, N_KC, JG], BF16,
                         kind="ExternalInput").ap()
    woT = nc.dram_tensor("woT", [128, N_JT, DIM], BF16,
                         kind="ExternalInput").ap()
    bq = nc.dram_tensor("bq", [128, N_JT], F32, kind="ExternalInput").ap()
    bk = nc.dram_tensor("bk", [128, N_JT], F32, kind="ExternalInput").ap()
    bvr = nc.dram_tensor("bvr", [128, JG], F32, kind="ExternalInput").ap()
    outT = nc.dram_tensor("outT", [DIM // 128, N_QC, 128, QC], F32,
                          kind="ExternalOutput").ap()

    with tile.TileContext(nc) as tc:
        with (
            tc.tile_pool(name="wproj", bufs=2) as wpool,
            tc.tile_pool(name="wo", bufs=1) as wopool,
            tc.tile_pool(name="xin", bufs=4) as xpool,
            tc.tile_pool(name="bias", bufs=1) as bpool,
            tc.tile_pool(name="qk", bufs=1) as qkpool,
            tc.tile_pool(name="vp", bufs=1) as vpool,
            tc.tile_pool(name="attn", bufs=2) as apool,
            tc.tile_pool(name="exp", bufs=2) as epool,
            tc.tile_pool(name="small", bufs=3) as spool,
            tc.tile_pool(name="outsb", bufs=4) as opool,
        ):
            # ---- persistent SBUF residents ----
            qhT = qkpool.tile([128, N_JT, S], BF16, tag="qhT")
            khT = qkpool.tile([128, N_JT, S], BF16, tag="khT")
            v_sb = vpool.tile([128, N_SJT, HPG, DK + 1], BF16, tag="v")
            wo_sb = wopool.tile([128, N_JT, DIM], BF16, tag="wo")
            bq_sb = bpool.tile([128, N_JT], F32, tag="bq")
            bk_sb = bpool.tile([128, N_JT], F32, tag="bk")
            bvr_sb = bpool.tile([128, JG], F32, tag="bvr")

            wk_sb = wpool.tile([128, N_KC, JG], BF16, tag="w", name="wk_sb")
            wq_sb = wpool.tile([128, N_KC, JG], BF16, tag="w", name="wq_sb")
            wv_sb = wpool.tile([128, N_KC, JG], BF16, tag="w", name="wv_sb")
            # wk split per contraction chunk: the first k matmul only needs
            # chunk 0, so it unblocks early
            for _kc in range(N_KC):
                nc.scalar.dma_start(wk_sb[:, _kc, :], wkT[:, _kc, :])
            nc.scalar.dma_start(wq_sb[:], wqT[:])
            nc.scalar.dma_start(wv_sb[:], wvT[:])
            nc.sync.dma_start(bq_sb[:], bq[:])
            nc.sync.dma_start(bk_sb[:], bk[:])
            nc.sync.dma_start(bvr_sb[:], bvr[:])
            # ones column for the softmax denominators
            nc.vector.memset(v_sb[:, :, :, DK:DK + 1], 1.0)
            # touch Exp early so the ACT table set loads during phase A
            warm = bpool.tile([1, 2], F32, tag="warm")
            nc.vector.memset(warm[:], 0.0)
            nc.scalar.activation(warm[:], warm[:],
                                 mybir.ActivationFunctionType.Exp)

            # ---- phase A head: K projection + Q first half ----
            # V projection and Q second half are deferred into the early
            # attention windows as filler work, so ScalarE starts exp ~40us
            # earlier. x loads batched: one 2MB DMA per (input, seq-half) so
            # the ~2us per-DMA completion latency amortizes over 8 kc chunks.
            fillers = []
            if "A" in phases:
             with tc.tile_pool(name="psA", bufs=4, space="PSUM") as psA:
                # head: full K projection, then just the Q slice the first
                # two windows need (head pairs 0-1, queries 0-511)
                xtq0 = None
                for sh in range(2):
                    ps2 = [psA.tile([128, 2, 512], F32, tag="ps",
                                    name=f"ps2_{i}") for i in range(4)]
                    xt = xpool.tile([128, N_KC, 1024], BF16, tag="x")
                    if sh == 0:
                        # split the very first load so the kc=0 matmuls
                        # start after 256KB instead of the full 2MB
                        nc.sync.dma_start(xt[:, 0, :], xkT[0, 0])
                        nc.sync.dma_start(
                            xt[:, 1:, :],
                            xkT[0].rearrange("k p s -> p k s")[:, 1:, :])
                    else:
                        nc.sync.dma_start(
                            xt[:], xkT[sh].rearrange("k p s -> p k s"))
                    if sh == 1:
                        # overlap the xq0 load with K's second-half matmuls
                        xtq0 = xpool.tile([128, N_KC, 1024], BF16, tag="x")
                        nc.sync.dma_start(
                            xtq0[:], xqT[0].rearrange("k p s -> p k s"))
                    for kc in range(N_KC):
                        for jt in range(N_JT):
                            for sc in range(2):
                                nc.tensor.matmul(
                                    ps2[jt][:, sc, :],
                                    wk_sb[:, kc, jt * 128:(jt + 1) * 128],
                                    xt[:, kc, sc * 512:(sc + 1) * 512],
                                    start=(kc == 0), stop=(kc == N_KC - 1))
                    for jt in range(N_JT):
                        nc.vector.tensor_scalar_add(
                            khT[:, jt, sh * 1024:(sh + 1) * 1024],
                            ps2[jt][:].rearrange("p a b -> p (a b)"),
                            bk_sb[:, jt:jt + 1])
                ps2 = [psA.tile([128, 2, 512], F32, tag="ps",
                                name=f"ps2_{i}") for i in range(2)]
                for kc in range(N_KC):
                    for jt in range(2):
                        nc.tensor.matmul(
                            ps2[jt][:, 0, :],
                            wq_sb[:, kc, jt * 128:(jt + 1) * 128],
                            xtq0[:, kc, 0:512],
                            start=(kc == 0), stop=(kc == N_KC - 1))
                for jt in range(2):
                    nc.vector.tensor_scalar_add(
                        qhT[:, jt, 0:512], ps2[jt][:, 0, :],
                        bq_sb[:, jt:jt + 1])

                # deferred inputs: V (both halves) and the rest of Q are
                # projected inside the early attention windows (fillers)
                xtv = []
                for sh in range(2):
                    xt = xpool.tile([128, N_KC, 1024], BF16, tag="x")
                    nc.sync.dma_start(
                        xt[:], xvT[sh].rearrange("k p s -> p k s"))
                    xtv.append(xt)
                xtq1 = xpool.tile([128, N_KC, 1024], BF16, tag="x")
                nc.sync.dma_start(
                    xtq1[:], xqT[1].rearrange("k p s -> p k s"))

                def v_thunk(st, pool):
                    sh, st8 = st // 8, st % 8
                    psv = pool.tile([128, 512], F32, tag="pj", name="pj")
                    for kc in range(N_KC):
                        nc.tensor.matmul(
                            psv[:],
                            xtv[sh][:, kc, st8 * 128:(st8 + 1) * 128],
                            wv_sb[:, kc, :],
                            start=(kc == 0), stop=(kc == N_KC - 1))
                    nc.vector.tensor_tensor(
                        v_sb[:, st, :, 0:DK],
                        psv[:].rearrange("p (h d) -> p h d", h=HPG),
                        bvr_sb[:, :].rearrange("p (h d) -> p h d", h=HPG),
                        mybir.AluOpType.add)

                def q_thunk(jt, sh, sc, pool):
                    xt = xtq0 if sh == 0 else xtq1
                    psq = pool.tile([128, 512], F32, tag="pj", name="pj")
                    for kc in range(N_KC):
                        nc.tensor.matmul(
                            psq[:],
                            wq_sb[:, kc, jt * 128:(jt + 1) * 128],
                            xt[:, kc, sc * 512:(sc + 1) * 512],
                            start=(kc == 0), stop=(kc == N_KC - 1))
                    nc.vector.tensor_scalar_add(
                        qhT[:, jt,
                            sh * 1024 + sc * 512:sh * 1024 + (sc + 1) * 512],
                        psq[:], bq_sb[:, jt:jt + 1])

                # order by deadline: Q for windows (0,2)/(0,3) first, then V
                # (needed by the first deferred attn@V at the end of window
                # (0,1)), then the remaining Q slices
                for jt in (2, 3):
                    fillers.append(("q", jt, 0, 0))
                for st in range(N_SJT):
                    fillers.append(("v", st))
                for jt in range(N_JT):
                    fillers.append(("q", jt, 0, 1))
                for jt in range(N_JT):
                    for sc in range(2):
                        fillers.append(("q", jt, 1, sc))

            # wo is first needed in phase C — load after projection weights
            nc.scalar.dma_start(wo_sb[:], woT[:])

            # ---- phases B/C: attention + out-projection, per query chunk ----
            # Pair pipeline: window (qc, hp) computes scores+exp for head
            # pair hp (the two heads issue as back-to-back matmuls on row
            # groups 0-63/64-127, running concurrently in the PE array) and
            # interleaves the DEFERRED attn@V of the previous pair (whose
            # et is fully staged in SBUF). Deferring attn@V lets the A/B
            # accumulations share one PSUM bank sequentially.
            if "B" in phases:
             with (
                tc.tile_pool(name="psS", bufs=2, space="PSUM") as psS,
                tc.tile_pool(name="psPA", bufs=2, space="PSUM") as psPA,
                tc.tile_pool(name="psPO", bufs=1, space="PSUM") as psPO,
                tc.tile_pool(name="psPJ", bufs=1, space="PSUM") as psPJ,
            ):
                grp = list(range(0, N_SJT, SJ_GRP)) + [N_SJT]
                n_grp = len(grp) - 1

                def run_filler(f):
                    if f[0] == "v":
                        v_thunk(f[1], psPJ)
                    else:
                        q_thunk(f[1], f[2], f[3], psPJ)

                def outproj(at_tile, qc_idx, ct, pool=None):
                    # alternate between psPO and psPJ (free once the fillers
                    # finish) so consecutive out-projections don't serialize
                    # on the PSUM->SBUF copy
                    if pool is None or pool is psPO:
                        po = psPO.tile([128, QC], F32, tag="po", name="po")
                    else:
                        po = pool.tile([128, 512], F32, tag="pj",
                                       name="pj")[:, :QC]
                    for jc in range(N_JT):
                        nc.tensor.matmul(
                            po[:],
                            wo_sb[:, jc, ct * 128:(ct + 1) * 128],
                            at_tile[:, jc, :],
                            start=(jc == 0), stop=(jc == N_JT - 1))
                    ob = opool.tile([128, QC], F32, tag="ob", name="ob")
                    nc.vector.tensor_copy(ob[:], po[:])
                    nc.sync.dma_start(outT[ct, qc_idx], ob[:])

                def issue_attnv(pv, k0, k1):
                    # deferred attn@V matmuls k0..k1 of pair pv
                    # (k < N_SJT: head A tiles; k >= N_SJT: head B tiles)
                    for kk in range(k0, k1):
                        h2, sjt = kk // N_SJT, kk % N_SJT
                        nc.tensor.matmul(
                            pv["pa"][:, h2, :],
                            v_sb[:, sjt, 2 * pv["hp"] + h2, :],
                            pv["et"][:, h2, sjt, :],
                            start=(sjt == 0), stop=(sjt == N_SJT - 1))

                def finish_pair(pv):
                    pa, hp = pv["pa"], pv["hp"]
                    den = spool.tile([1, 2, QC], F32, tag="den")
                    bc = spool.tile([DK, 2, QC], F32, tag="bc")
                    for h2 in range(2):
                        nc.vector.reciprocal(den[:, h2], pa[DK:DK + 1, h2])
                        nc.gpsimd.partition_broadcast(bc[:, h2], den[:, h2])
                        nc.vector.tensor_tensor(
                            pv["at"][h2 * 64:h2 * 64 + 64, hp, :],
                            pa[:DK, h2], bc[:, h2],
                            mybir.AluOpType.mult)

                at_tiles = {}
                pending = []
                prev = None
                for qc in range(N_QC):
                    si = slice(qc * QC, (qc + 1) * QC)
                    at_sb = apool.tile([128, N_JT, QC], BF16, tag="at")
                    at_tiles[qc] = at_sb
                    for hp in range(N_JT):
                        et = epool.tile([128, 2, N_SJT, QC], BF16, tag="e")
                        pa = psPA.tile([128, 2, QC], F32, tag="pa",
                                       name="pa")[:DK + 1]
                        nmm_prev = 2 * N_SJT if prev is not None else 0
                        # first deferred pair: V is still being projected by
                        # the fillers, so issue its attn@V only at window end
                        defer_all = qc == 0 and hp == 1
                        done = 0
                        for gi in range(n_grp):
                            g0, g1 = grp[gi], grp[gi + 1]
                            gn = g1 - g0
                            ps = psS.tile([128, 2, SJ_GRP, QC], F32,
                                          tag="ps")
                            # row range for the second head of the pair:
                            # 64:128 normally (concurrent row groups);
                            # unpack_probe forces 0:64 (serialized, timing
                            # probe only — results wrong for head B)
                            pb = 0 if unpack_probe else 64
                            for i in range(gn):
                                sjt = g0 + i
                                ks = slice(sjt * 128, (sjt + 1) * 128)
                                nc.tensor.matmul(
                                    ps[:, 0, i, :], khT[0:64, hp, ks],
                                    qhT[0:64, hp, si],
                                    start=True, stop=True)
                                nc.tensor.matmul(
                                    ps[:, 1, i, :], khT[pb:pb + 64, hp, ks],
                                    qhT[pb:pb + 64, hp, si],
                                    start=True, stop=True)
                            nc.scalar.activation(
                                et[:, :, g0:g1, :], ps[:, :, :gn, :],
                                mybir.ActivationFunctionType.Exp,
                                scale=1.0 / np.sqrt(DK))
                            # leftover projection work rides under the
                            # pipeline-fill windows
                            for _ in range(2):
                                if fillers:
                                    run_filler(fillers.pop(0))
                            # deferred attn@V rides under this pair's exp,
                            # front-loaded one group so the window tail has
                            # no matmuls between the last exp and the next
                            # window's scores (measured ~0.7us ACT stall per
                            # window otherwise)
                            tgt = 0 if defer_all else \
                                min(nmm_prev, nmm_prev * (gi + 2) // n_grp)
                            if prev is not None and tgt > done:
                                issue_attnv(prev, done, tgt)
                                done = tgt

                            if "C" in phases and gi in (2, 5) and pending:
                                outproj(*pending.pop(0),
                                        psPJ if gi == 5 else psPO)
                        if prev is not None:
                            if done < nmm_prev:
                                issue_attnv(prev, done, nmm_prev)
                            finish_pair(prev)
                        prev = {"hp": hp, "et": et, "pa": pa, "at": at_sb}
                        if hp == 0 and qc > 0 and "C" in phases:
                            for ct in range(DIM // 128):
                                pending.append((at_tiles[qc - 1], qc - 1, ct))
                # flush: last pair's attn@V + remaining out-projections
                if prev is not None:
                    issue_attnv(prev, 0, 2 * N_SJT)
                    finish_pair(prev)
                if "C" in phases:
                    for ct in range(DIM // 128):
                        pending.append((at_tiles[N_QC - 1], N_QC - 1, ct))
                    for i in range(len(pending)):
                        outproj(*pending[i], psPJ if i % 2 else psPO)
                    pending.clear()
            if "B" not in phases:
                with tc.tile_pool(name="fb", bufs=1) as fb:
                    t0 = fb.tile([128, QC], F32)
                    nc.vector.memset(t0[:], 0.0)
                    nc.sync.dma_start(outT[0, 0], t0[:])
    nc.compile()
    return nc


_CACHED_NC = None


def _get_program():
    global _CACHED_NC
    if _CACHED_NC is None:
        _CACHED_NC = build_program()
    return _CACHED_NC


def _make_in_maps(q, k, v, Wq, bq, Wk, bk, Wv, bv, Wo, bo):
    f32 = np.float32

    def chunk_x(x):
        # [S, DIM] -> transposed, pre-chunked [2, N_KC, 128, 1024] bf16
        xT = np.asarray(x, f32).T.astype(bfloat16)   # [DIM, S]
        return np.ascontiguousarray(
            xT.reshape(N_KC, 128, 2, 1024).transpose(2, 0, 1, 3))

    in_maps = []
    # per-batch transposed activations (shared between the 2 TP cores)
    xT = {}
    for b in range(B):
        xT[b] = (chunk_x(q[b]), chunk_x(k[b]), chunk_x(v[b]))
    wg = {}
    for g in range(2):
        js = slice(g * JG, (g + 1) * JG)

        def tile_w(W):
            # W[js, :].T = [DIM, JG] -> [128, N_KC, JG] bf16
            wT = np.asarray(W, f32)[js, :].T.astype(bfloat16)
            return np.ascontiguousarray(
                wT.reshape(N_KC, 128, JG).transpose(1, 0, 2))

        woT_g = np.asarray(Wo, f32)[:, js].T.astype(bfloat16)   # [JG, DIM]
        wg[g] = {
            "wqT": tile_w(Wq),
            "wkT": tile_w(Wk),
            "wvT": tile_w(Wv),
            "woT": np.ascontiguousarray(
                woT_g.reshape(N_JT, 128, DIM).transpose(1, 0, 2)),
            "bq": np.ascontiguousarray(
                np.asarray(bq, f32)[js].reshape(N_JT, 128).T),
            "bk": np.ascontiguousarray(
                np.asarray(bk, f32)[js].reshape(N_JT, 128).T),
            "bvr": np.ascontiguousarray(
                np.broadcast_to(np.asarray(bv, f32)[js], (128, JG))),
        }
    for c in range(N_CORES):
        b, g = c // 2, c % 2
        m = {"xqT": xT[b][0], "xkT": xT[b][1], "xvT": xT[b][2]}
        m.update(wg[g])
        in_maps.append(m)
    return in_maps


def _gather(results, bo):
    out = np.empty((B, S, DIM), np.float32)
    bo32 = np.asarray(bo, np.float32)
    for b in range(B):
        acc = results[2 * b]["outT"] + results[2 * b + 1]["outT"]
        # [ct, qc, p, s'] -> [DIM, S]
        full = acc.transpose(0, 2, 1, 3).reshape(DIM, S)
        out[b] = full.T + bo32
    return out


def kernel(q, k, v, Wq, bq, Wk, bk, Wv, bv, Wo, bo):
    import time as _time
    nc = _get_program()
    in_maps = _make_in_maps(q, k, v, Wq, bq, Wk, bk, Wv, bv, Wo, bo)
    last_err = None
    for attempt in range(3):
        try:
            res = run_bass_kernel_spmd(nc, in_maps,
                                       core_ids=list(range(N_CORES)))
            return _gather(res.results, bo)
        except Exception as e:  # transient device/tunnel errors
            last_err = e
            _time.sleep(20 * (attempt + 1))
    raise last_err


# revision 29
# speedup vs baseline: 1.1491x; 1.1491x over previous
"""Multi-head attention (B=4, S=2048, D=1024, H=16) on 8 Trainium2 cores.

Sharding: data-parallel over the 4 batches x tensor-parallel over 2 groups
of 8 heads. Core c handles batch c//2, head group c%2. Each core computes
its group's slice of the out-projection; the host sums the two partial
outputs per batch.

All matmul operands are bf16 (fp32 PSUM accumulation); rel-err budget is
2e-2 so bf16 rounding (~0.4%) is fine and it halves both PE streaming and
DMA cost vs float32r.

Device-side layout (per core):
  qhT/khT [128, 4, S] bf16 : projections transposed (head-pair dim on
                     partitions: head 2j at partitions 0-63, head 2j+1 at
                     64-127; sequence on free dim).
  scores  : per head pair, key tiles on partitions, 256-query chunks; the
            two heads of a pair issue as back-to-back matmuls on row groups
            (0,0)/(64,0) so they run concurrently in the PE array.
  softmax : exp on ScalarE straight out of PSUM in [128, 3, 2, 256] groups
            (1536 elem/instr); denominators from a ones column appended to
            V during the attn@V accumulation.
  outT [8, 8, 128, 256] f32 : transposed partial out-projection, summed on
            host.
"""
import sys

for _p in ("/opt/trn_rl_repo", "/root/.axon_site/_ro/trn_rl_repo"):
    if _p not in sys.path:
        sys.path.append(_p)

import numpy as np
from ml_dtypes import bfloat16

import concourse.bass as bass
import concourse.tile as tile
from concourse import bacc, mybir
from concourse.bass_utils import run_bass_kernel_spmd

N_CORES = 8
B, S, DIM, H, DK = 4, 2048, 1024, 16, 64
JG = DIM // 2          # head-group width (8 heads x 64)
HPG = 8                # heads per group
BF16 = mybir.dt.bfloat16
F32 = mybir.dt.float32

N_KC = DIM // 128      # contraction chunks for projections
N_JT = JG // 128       # 128-row tiles of the group width
N_SJT = S // 128       # key tiles
QC = 256               # queries per attention chunk
N_QC = S // QC         # attention chunks
SJ_GRP = 2             # key tiles per score/exp group (2 PSUM banks)


def build_program(phases="ABC", unpack_probe=False):
    nc = bacc.Bacc("TRN2", target_bir_lowering=False, debug=False,
                   num_devices=N_CORES)
    xqT = nc.dram_tensor("xqT", [2, N_KC, 128, 1024], BF16,
                         kind="ExternalInput").ap()
    xkT = nc.dram_tensor("xkT", [2, N_KC, 128, 1024], BF16,
                         kind="ExternalInput").ap()
    xvT = nc.dram_tensor("xvT", [2, N_KC, 128, 1024], BF16,
                         kind="ExternalInput").ap()
    wqT = nc.dram_tensor("wqT", [128, N_KC, JG], BF16,
                         kind="ExternalInput").ap()
    wkT = nc.dram_tensor("wkT", [128, N_KC, JG], BF16,
                         kind="ExternalInput").ap()
    wvT = nc.dram_tensor("wvT", [128, N_KC, JG], BF16,
                         kind="ExternalInput").ap()
    woT = nc.dram_tensor("woT", [128, N_JT, DIM], BF16,
                         kind="ExternalInput").ap()
    bq = nc.dram_tensor("bq", [128, N_JT], F32, kind="ExternalInput").ap()
    bk = nc.dram_tensor("bk", [128, N_JT], F32, kind="ExternalInput").ap()
    bvr = nc.dram_tensor("bvr", [128, JG], F32, kind="ExternalInput").ap()
    outT = nc.dram_tensor("outT", [DIM // 128, N_QC, 128, QC], F32,
                          kind="ExternalOutput").ap()

    with tile.TileContext(nc) as tc:
        with (
            tc.tile_pool(name="wproj", bufs=2) as wpool,
            tc.tile_pool(name="wo", bufs=1) as wopool,
            tc.tile_pool(name="xin", bufs=4) as xpool,
            tc.tile_pool(name="bias", bufs=1) as bpool,
            tc.tile_pool(name="qk", bufs=1) as qkpool,
            tc.tile_pool(name="vp", bufs=1) as vpool,
            tc.tile_pool(name="attn", bufs=2) as apool,
            tc.tile_pool(name="exp", bufs=2) as epool,
            tc.tile_pool(name="small", bufs=3) as spool,
            tc.tile_pool(name="outsb", bufs=4) as opool,
        ):
            # ---- persistent SBUF residents ----
            qhT = qkpool.tile([128, N_JT, S], BF16, tag="qhT")
            khT = qkpool.tile([128, N_JT, S], BF16, tag="khT")
            v_sb = vpool.tile([128, N_SJT, HPG, DK + 1], BF16, tag="v")
            wo_sb = wopool.tile([128, N_JT, DIM], BF16, tag="wo")
            bq_sb = bpool.tile([128, N_JT], F32, tag="bq")
            bk_sb = bpool.tile([128, N_JT], F32, tag="bk")
            bvr_sb = bpool.tile([128, JG], F32, tag="bvr")

            wk_sb = wpool.tile([128, N_KC, JG], BF16, tag="w", name="wk_sb")
            wq_sb = wpool.tile([128, N_KC, JG], BF16, tag="w", name="wq_sb")
            wv_sb = wpool.tile([128, N_KC, JG], BF16, tag="w", name="wv_sb")
            # wk split per contraction chunk: the first k matmul only needs
            # chunk 0, so it unblocks early
            for _kc in range(N_KC):
                nc.scalar.dma_start(wk_sb[:, _kc, :], wkT[:, _kc, :])
            nc.scalar.dma_start(wq_sb[:], wqT[:])
            nc.scalar.dma_start(wv_sb[:], wvT[:])
            nc.sync.dma_start(bq_sb[:], bq[:])
            nc.sync.dma_start(bk_sb[:], bk[:])
            nc.sync.dma_start(bvr_sb[:], bvr[:])
            # ones column for the softmax denominators
            nc.vector.memset(v_sb[:, :, :, DK:DK + 1], 1.0)
            # touch Exp early so the ACT table set loads during phase A
            warm = bpool.tile([1, 2], F32, tag="warm")
            nc.vector.memset(warm[:], 0.0)
            nc.scalar.activation(warm[:], warm[:],
                                 mybir.ActivationFunctionType.Exp)

            # ---- phase A head: K projection + Q first half ----
            # V projection and Q second half are deferred into the early
            # attention windows as filler work, so ScalarE starts exp ~40us
            # earlier. x loads batched: one 2MB DMA per (input, seq-half) so
            # the ~2us per-DMA completion latency amortizes over 8 kc chunks.
            fillers = []
            if "A" in phases:
             with tc.tile_pool(name="psA", bufs=4, space="PSUM") as psA:
                # head: full K projection, then just the Q slice the first
                # two windows need (head pairs 0-1, queries 0-511)
                xtq0 = None
                for sh in range(2):
                    ps2 = [psA.tile([128, 2, 512], F32, tag="ps",
                                    name=f"ps2_{i}") for i in range(4)]
                    xt = xpool.tile([128, N_KC, 1024], BF16, tag="x")
                    if sh == 0:
                        # split the very first load so the kc=0 matmuls
                        # start after 256KB instead of the full 2MB
                        nc.sync.dma_start(xt[:, 0, :], xkT[0, 0])
                        nc.sync.dma_start(
                            xt[:, 1:, :],
                            xkT[0].rearrange("k p s -> p k s")[:, 1:, :])
                    else:
                        nc.sync.dma_start(
                            xt[:], xkT[sh].rearrange("k p s -> p k s"))
                    if sh == 1:
                        # overlap the xq0 load with K's second-half matmuls
                        xtq0 = xpool.tile([128, N_KC, 1024], BF16, tag="x")
                        nc.sync.dma_start(
                            xtq0[:], xqT[0].rearrange("k p s -> p k s"))
                    for kc in range(N_KC):
                        for jt in range(N_JT):
                            for sc in range(2):
                                nc.tensor.matmul(
                                    ps2[jt][:, sc, :],
                                    wk_sb[:, kc, jt * 128:(jt + 1) * 128],
                                    xt[:, kc, sc * 512:(sc + 1) * 512],
                                    start=(kc == 0), stop=(kc == N_KC - 1))
                    for jt in range(N_JT):
                        nc.vector.tensor_scalar_add(
                            khT[:, jt, sh * 1024:(sh + 1) * 1024],
                            ps2[jt][:].rearrange("p a b -> p (a b)"),
                            bk_sb[:, jt:jt + 1])
                ps2 = [psA.tile([128, 2, 512], F32, tag="ps",
                                name=f"ps2_{i}") for i in range(2)]
                for kc in range(N_KC):
                    for jt in range(2):
                        nc.tensor.matmul(
                            ps2[jt][:, 0, :],
                            wq_sb[:, kc, jt * 128:(jt + 1) * 128],
                            xtq0[:, kc, 0:512],
                            start=(kc == 0), stop=(kc == N_KC - 1))
                for jt in range(2):
                    nc.vector.tensor_scalar_add(
                        qhT[:, jt, 0:512], ps2[jt][:, 0, :],
                        bq_sb[:, jt:jt + 1])

                # deferred inputs: V (both halves) and the rest of Q are
                # projected inside the early attention windows (fillers)
                xtv = []
                for sh in range(2):
                    xt = xpool.tile([128, N_KC, 1024], BF16, tag="x")
                    nc.sync.dma_start(
                        xt[:], xvT[sh].rearrange("k p s -> p k s"))
                    xtv.append(xt)
                xtq1 = xpool.tile([128, N_KC, 1024], BF16, tag="x")
                nc.sync.dma_start(
                    xtq1[:], xqT[1].rearrange("k p s -> p k s"))

                def v_thunk(st, pool):
                    sh, st8 = st // 8, st % 8
                    psv = pool.tile([128, 512], F32, tag="pj", name="pj")
                    for kc in range(N_KC):
                        nc.tensor.matmul(
                            psv[:],
                            xtv[sh][:, kc, st8 * 128:(st8 + 1) * 128],
                            wv_sb[:, kc, :],
                            start=(kc == 0), stop=(kc == N_KC - 1))
                    nc.vector.tensor_tensor(
                        v_sb[:, st, :, 0:DK],
                        psv[:].rearrange("p (h d) -> p h d", h=HPG),
                        bvr_sb[:, :].rearrange("p (h d) -> p h d", h=HPG),
                        mybir.AluOpType.add)

                def q_thunk(jt, sh, sc, pool):
                    xt = xtq0 if sh == 0 else xtq1
                    psq = pool.tile([128, 512], F32, tag="pj", name="pj")
                    for kc in range(N_KC):
                        nc.tensor.matmul(
                            psq[:],
                            wq_sb[:, kc, jt * 128:(jt + 1) * 128],
                            xt[:, kc, sc * 512:(sc + 1) * 512],
                            start=(kc == 0), stop=(kc == N_KC - 1))
                    nc.vector.tensor_scalar_add(
                        qhT[:, jt,
                            sh * 1024 + sc * 512:sh * 1024 + (sc + 1) * 512],
                        psq[:], bq_sb[:, jt:jt + 1])

                # order by deadline: Q for windows (0,2)/(0,3) first, then V
                # (needed by the first deferred attn@V at the end of window
                # (0,1)), then the remaining Q slices
                for jt in (2, 3):
                    fillers.append(("q", jt, 0, 0))
                for st in range(N_SJT):
                    fillers.append(("v", st))
                for jt in range(N_JT):
                    fillers.append(("q", jt, 0, 1))
                for jt in range(N_JT):
                    for sc in range(2):
                        fillers.append(("q", jt, 1, sc))

            # wo is first needed in phase C — load after projection weights
            nc.scalar.dma_start(wo_sb[:], woT[:])

            # ---- phases B/C: attention + out-projection, per query chunk ----
            # Pair pipeline: window (qc, hp) computes scores+exp for head
            # pair hp (the two heads issue as back-to-back matmuls on row
            # groups 0-63/64-127, running concurrently in the PE array) and
            # interleaves the DEFERRED attn@V of the previous pair (whose
            # et is fully staged in SBUF). Deferring attn@V lets the A/B
            # accumulations share one PSUM bank sequentially.
            if "B" in phases:
             with (
                tc.tile_pool(name="psS", bufs=2, space="PSUM") as psS,
                tc.tile_pool(name="psPA", bufs=2, space="PSUM") as psPA,
                tc.tile_pool(name="psPO", bufs=1, space="PSUM") as psPO,
                tc.tile_pool(name="psPJ", bufs=1, space="PSUM") as psPJ,
            ):
                grp = list(range(0, N_SJT, SJ_GRP)) + [N_SJT]
                n_grp = len(grp) - 1

                def run_filler(f):
                    if f[0] == "v":
                        v_thunk(f[1], psPJ)
                    else:
                        q_thunk(f[1], f[2], f[3], psPJ)

                def outproj(at_tile, qc_idx, ct, pool=None):
                    # alternate between psPO and psPJ (free once the fillers
                    # finish) so consecutive out-projections don't serialize
                    # on the PSUM->SBUF copy
                    if pool is None or pool is psPO:
                        po = psPO.tile([128, QC], F32, tag="po", name="po")
                    else:
                        po = pool.tile([128, 512], F32, tag="pj",
                                       name="pj")[:, :QC]
                    for jc in range(N_JT):
                        nc.tensor.matmul(
                            po[:],
                            wo_sb[:, jc, ct * 128:(ct + 1) * 128],
                            at_tile[:, jc, :],
                            start=(jc == 0), stop=(jc == N_JT - 1))
                    ob = opool.tile([128, QC], F32, tag="ob", name="ob")
                    nc.vector.tensor_copy(ob[:], po[:])
                    nc.sync.dma_start(outT[ct, qc_idx], ob[:])

                def issue_attnv(pv, k0, k1):
                    # deferred attn@V matmuls k0..k1 of pair pv
                    # (k < N_SJT: head A tiles; k >= N_SJT: head B tiles)
                    for kk in range(k0, k1):
                        h2, sjt = kk // N_SJT, kk % N_SJT
                        nc.tensor.matmul(
                            pv["pa"][:, h2, :],
                            v_sb[:, sjt, 2 * pv["hp"] + h2, :],
                            pv["et"][:, h2, sjt, :],
                            start=(sjt == 0), stop=(sjt == N_SJT - 1))

                def finish_pair(pv):
                    pa, hp = pv["pa"], pv["hp"]
                    den = spool.tile([1, 2, QC], F32, tag="den")
                    bc = spool.tile([DK, 2, QC], F32, tag="bc")
                    for h2 in range(2):
                        nc.vector.reciprocal(den[:, h2], pa[DK:DK + 1, h2])
                        nc.gpsimd.partition_broadcast(bc[:, h2], den[:, h2])
                        nc.vector.tensor_tensor(
                            pv["at"][h2 * 64:h2 * 64 + 64, hp, :],
                            pa[:DK, h2], bc[:, h2],
                            mybir.AluOpType.mult)

                at_tiles = {}
                pending = []
                prev = None
                for qc in range(N_QC):
                    si = slice(qc * QC, (qc + 1) * QC)
                    at_sb = apool.tile([128, N_JT, QC], BF16, tag="at")
                    at_tiles[qc] = at_sb
                    for hp in range(N_JT):
                        et = epool.tile([128, 2, N_SJT, QC], BF16, tag="e")
                        pa = psPA.tile([128, 2, QC], F32, tag="pa",
                                       name="pa")[:DK + 1]
                        nmm_prev = 2 * N_SJT if prev is not None else 0
                        # first deferred pair: V is still being projected by
                        # the fillers, so issue its attn@V only at window end
                        defer_all = qc == 0 and hp == 1
                        done = 0
                        for gi in range(n_grp):
                            g0, g1 = grp[gi], grp[gi + 1]
                            gn = g1 - g0
                            ps = psS.tile([128, 2, SJ_GRP, QC], F32,
                                          tag="ps")
                            # row range for the second head of the pair:
                            # 64:128 normally (concurrent row groups);
                            # unpack_probe forces 0:64 (serialized, timing
                            # probe only — results wrong for head B)
                            pb = 0 if unpack_probe else 64
                            for i in range(gn):
                                sjt = g0 + i
                                ks = slice(sjt * 128, (sjt + 1) * 128)
                                nc.tensor.matmul(
                                    ps[:, 0, i, :], khT[0:64, hp, ks],
                                    qhT[0:64, hp, si],
                                    start=True, stop=True)
                                nc.tensor.matmul(
                                    ps[:, 1, i, :], khT[pb:pb + 64, hp, ks],
                                    qhT[pb:pb + 64, hp, si],
                                    start=True, stop=True)
                            nc.scalar.activation(
                                et[:, :, g0:g1, :], ps[:, :, :gn, :],
                                mybir.ActivationFunctionType.Exp,
                                scale=1.0 / np.sqrt(DK))
                            # leftover projection work rides under the
                            # pipeline-fill windows: deadline-critical thunks
                            # (Q for windows (0,2)/(0,3), V) go 2-per-group;
                            # the rest trickle 1-per-3-groups into the
                            # ACT-bound steady windows
                            n_pop = 2 if len(fillers) > 12 else \
                                (1 if gi % 3 == 0 else 0)
                            for _ in range(n_pop):
                                if fillers:
                                    run_filler(fillers.pop(0))
                            # deferred attn@V rides under this pair's exp,
                            # front-loaded one group so the window tail has
                            # no matmuls between the last exp and the next
                            # window's scores (measured ~0.7us ACT stall per
                            # window otherwise)
                            tgt = 0 if defer_all else \
                                min(nmm_prev, nmm_prev * (gi + 2) // n_grp)
                            if prev is not None and tgt > done:
                                issue_attnv(prev, done, tgt)
                                done = tgt

                            if "C" in phases and gi in (2, 5) and pending:
                                outproj(*pending.pop(0),
                                        psPJ if gi == 5 else psPO)
                        if prev is not None:
                            if done < nmm_prev:
                                issue_attnv(prev, done, nmm_prev)
                            finish_pair(prev)
                        prev = {"hp": hp, "et": et, "pa": pa, "at": at_sb}
                        if hp == 0 and qc > 0 and "C" in phases:
                            for ct in range(DIM // 128):
                                pending.append((at_tiles[qc - 1], qc - 1, ct))
                # flush: last pair's attn@V + remaining out-projections
                if prev is not None:
                    issue_attnv(prev, 0, 2 * N_SJT)
                    finish_pair(prev)
                if "C" in phases:
                    for ct in range(DIM // 128):
                        pending.append((at_tiles[N_QC - 1], N_QC - 1, ct))
                    for i in range(len(pending)):
                        outproj(*pending[i], psPJ if i % 2 else psPO)
                    pending.clear()
            if "B" not in phases:
                with tc.tile_pool(name="fb", bufs=1) as fb:
                    t0 = fb.tile([128, QC], F32)
                    nc.vector.memset(t0[:], 0.0)
                    nc.sync.dma_start(outT[0, 0], t0[:])
    nc.compile()
    return nc


_CACHED_NC = None


def _get_program():
    global _CACHED_NC
    if _CACHED_NC is None:
        _CACHED_NC = build_program()
    return _CACHED_NC


def _make_in_maps(q, k, v, Wq, bq, Wk, bk, Wv, bv, Wo, bo):
    f32 = np.float32

    def chunk_x(x):
        # [S, DIM] -> transposed, pre-chunked [2, N_KC, 128, 1024] bf16
        xT = np.asarray(x, f32).T.astype(bfloat16)   # [DIM, S]
        return np.ascontiguousarray(
            xT.reshape(N_KC, 128, 2, 1024).transpose(2, 0, 1, 3))

    in_maps = []
    # per-batch transposed activations (shared between the 2 TP cores)
    xT = {}
    for b in range(B):
        xT[b] = (chunk_x(q[b]), chunk_x(k[b]), chunk_x(v[b]))
    wg = {}
    for g in range(2):
        js = slice(g * JG, (g + 1) * JG)

        def tile_w(W):
            # W[js, :].T = [DIM, JG] -> [128, N_KC, JG] bf16
            wT = np.asarray(W, f32)[js, :].T.astype(bfloat16)
            return np.ascontiguousarray(
                wT.reshape(N_KC, 128, JG).transpose(1, 0, 2))

        woT_g = np.asarray(Wo, f32)[:, js].T.astype(bfloat16)   # [JG, DIM]
        wg[g] = {
            "wqT": tile_w(Wq),
            "wkT": tile_w(Wk),
            "wvT": tile_w(Wv),
            "woT": np.ascontiguousarray(
                woT_g.reshape(N_JT, 128, DIM).transpose(1, 0, 2)),
            "bq": np.ascontiguousarray(
                np.asarray(bq, f32)[js].reshape(N_JT, 128).T),
            "bk": np.ascontiguousarray(
                np.asarray(bk, f32)[js].reshape(N_JT, 128).T),
            "bvr": np.ascontiguousarray(
                np.broadcast_to(np.asarray(bv, f32)[js], (128, JG))),
        }
    for c in range(N_CORES):
        b, g = c // 2, c % 2
        m = {"xqT": xT[b][0], "xkT": xT[b][1], "xvT": xT[b][2]}
        m.update(wg[g])
        in_maps.append(m)
    return in_maps


def _gather(results, bo):
    out = np.empty((B, S, DIM), np.float32)
    bo32 = np.asarray(bo, np.float32)
    for b in range(B):
        acc = results[2 * b]["outT"] + results[2 * b + 1]["outT"]
        # [ct, qc, p, s'] -> [DIM, S]
        full = acc.transpose(0, 2, 1, 3).reshape(DIM, S)
        out[b] = full.T + bo32
    return out


def kernel(q, k, v, Wq, bq, Wk, bk, Wv, bv, Wo, bo):
    import time as _time
    nc = _get_program()
    in_maps = _make_in_maps(q, k, v, Wq, bq, Wk, bk, Wv, bv, Wo, bo)
    last_err = None
    for attempt in range(3):
        try:
            res = run_bass_kernel_spmd(nc, in_maps,
                                       core_ids=list(range(N_CORES)))
            return _gather(res.results, bo)
        except Exception as e:  # transient device/tunnel errors
            last_err = e
            _time.sleep(20 * (attempt + 1))
    raise last_err
